# revision 41
# baseline (speedup 1.0000x reference)
"""Trainium2 Bass kernel for nn_EndpointRegressor (2x TransformerConv GNN +
AttentionalAggregation) distributed over 8 NeuronCores.

Sharding: edges partitioned by destination node range (6272 nodes/core);
each core owns its dst nodes exclusively, so segment softmax/scatter stats
need no cross-core reduction.  Per layer each core computes k|v for its own
nodes, the k|v table is AllGather-ed, and each core gathers k|v rows for its
edge shard with dma_gather.  The segment softmax uses exp without max
subtraction (alpha range is ~±0.09 for this model family) and folds the
denominator division to the node side: out = (sum ex*v)/(sum ex).
Scatter-adds are one-hot matmuls accumulated in PSUM per 128-node window.

v2: all matmul/gather/DMA datapaths in bf16 (fp32 matmuls are 4 cyc/row on
TRN2 PE vs 1 for bf16 and forbid fast-weight-load); sigmoid computed via the
Exp table (beta = 1/(1+exp(-z))) so the ACT engine never swaps activation
tables inside the edge loop; kv rows padded to 384 elems (768B, dma_gather
requires elem bytes % 256 == 0).
"""
import contextlib
import math
import numpy as np
import ml_dtypes

bfloat16 = ml_dtypes.bfloat16

import concourse.bass as bass
import concourse.bacc as bacc
import concourse.mybir as mybir
import concourse.tile as tile
from concourse._compat import get_trn_type
from concourse.bass_utils import run_bass_kernel_spmd
from concourse.library_config import mlp

# ---- problem constants (fixed by the problem spec) ----
N, E, G = 50000, 500000, 32
H, D = 4, 40
HID = H * D            # 160
JK = 2 * HID           # 320
KVP = 384              # padded kv row (bf16: 768B, % 256 == 0)
NCORES = 8
NSHARD = 6272          # 49*128 nodes per core
NPAD = NCORES * NSHARD # 50176
WIN = NSHARD // 128    # 49
SPLIT = NPAD // 2      # 25088: low/high kv-table split (int16 gather indices)
GS = 1024              # slots per dma_gather instruction
INVSQD = 1.0 / math.sqrt(float(D))

f32 = mybir.dt.float32
bf16 = mybir.dt.bfloat16
i16 = mybir.dt.int16


def _wrap16(ix):
    """[n] int16 -> [128, n//16] dma_gather index layout (16-wrap, x8 replicate)."""
    return np.tile(ix.reshape(-1, 16).T, (8, 1))


def _preprocess(x, edge_index, edge_attr, batch):
    """Sort edges by dst, shard by dst range, split each window's edges into
    low/high src groups, pad to uniform chunk counts. Returns per-core input
    arrays + the adaptive chunk capacities (C_L, C_H)."""
    src = np.asarray(edge_index[0], dtype=np.int64)
    dst = np.asarray(edge_index[1], dtype=np.int64)
    ea = np.asarray(edge_attr, dtype=np.float32)
    order = np.argsort(dst, kind="stable")
    src, dst, ea = src[order], dst[order], ea[order]

    core = dst // NSHARD
    win = (dst % NSHARD) // 128
    low = src < SPLIT

    # bucket edge indices per (core, window, low/high)
    buckets = {}
    for r in range(NCORES):
        m_r = core == r
        for w in range(WIN):
            m = m_r & (win == w)
            idx = np.nonzero(m)[0]
            lo = idx[low[idx]]
            hi = idx[~low[idx]]
            buckets[(r, w)] = (lo, hi)

    C_L = max(1, max((len(b[0]) + 127) // 128 for b in buckets.values()))
    C_H = max(1, max((len(b[1]) + 127) // 128 for b in buckets.values()))
    NCH = C_L + C_H
    NGL = (WIN * C_L * 128 + GS - 1) // GS
    NGH = (WIN * C_H * 128 + GS - 1) // GS

    per_core = []
    for r in range(NCORES):
        Lslots = np.zeros(NGL * GS, np.int64)      # gather idx (low table)
        Hslots = np.zeros(NGH * GS, np.int64)
        # per-edge [ea,1] 5-vector replicated per head: [WIN, 128, NCH, 4, 5]
        eaR = np.zeros((WIN, 128, NCH, H, 5), np.float32)
        eaT = np.zeros((WIN, 5, NCH * 128), np.float32)
        eaT[:, 4, :] = 1.0
        dstrel = np.full((WIN, 128, NCH), -1.0, np.float32)  # host-only now
        for w in range(WIN):
            lo, hi = buckets[(r, w)]
            for (idx_e, slots, base_c, j0, table_off) in (
                (lo, Lslots, w * C_L, 0, 0),
                (hi, Hslots, w * C_H, C_L, SPLIT),
            ):
                n = len(idx_e)
                s0 = base_c * 128
                slots[s0 : s0 + n] = src[idx_e] - table_off
                # pipeline slot (w, j0 + k//128, k%128)
                kk = np.arange(n)
                jj = j0 + kk // 128
                pp = kk % 128
                ea5 = np.concatenate([ea[idx_e], np.ones((n, 1), np.float32)], 1)
                eaR[w, pp, jj] = ea5[:, None, :]
                eaT[w, :4, :][:, jj * 128 + pp] = ea[idx_e].T
                dstrel[w, pp, jj] = (dst[idx_e] % 128).astype(np.float32)
        # pad slots keep idx=0: every gather slot must be WRITTEN on hw
        # (unwritten SBUF can hold NaN garbage that poisons 0*NaN in the
        # scatter matmul), so no -1 skip sentinels.
        # own-node arrays
        n0 = r * NSHARD
        xT = np.zeros((5, NSHARD), np.float32)
        batchc = np.full((WIN, 128, 1), -1.0, np.float32)
        n_real = max(0, min(NSHARD, N - n0))
        if n_real > 0:
            xT[:, :n_real] = np.asarray(x[n0 : n0 + n_real], np.float32).T
            bc = np.asarray(batch[n0 : n0 + n_real], np.float32).reshape(-1, 1)
            batchc.reshape(NSHARD, 1)[:n_real] = bc
        # host-built one-hot scatter/gather matrices (bf16 streams):
        # stv[w, e, j, n] = (dst-in-window of edge slot (w,j,e) == n)
        iota = np.arange(128, dtype=np.float32)
        stv = dstrel[:, :, :, None] == iota          # [WIN, 128e, NCH, 128n]
        ssv = np.ascontiguousarray(stv.transpose(0, 3, 2, 1))  # [WIN, 128n, NCH, 128e]
        sse = np.concatenate([
            stv.reshape(WIN, 128, NCH * 128),
            ssv.reshape(WIN, 128, NCH * 128),
            eaR.reshape(WIN, 128, NCH * 20),
        ], axis=2).astype(bfloat16)
        per_core.append(
            dict(
                xT=xT.astype(bfloat16),
                idxL=np.ascontiguousarray(_wrap16(Lslots.astype(np.int16))),
                idxH=np.ascontiguousarray(_wrap16(Hslots.astype(np.int16))),
                sse=sse,
                eaT=eaT.astype(bfloat16),
                batchc=batchc,
            )
        )
    return per_core, C_L, C_H


def _weights(inp):
    """Host-side weight packing (bias folding, concat layouts)."""
    w = {}
    b_in = inp["b_in"].astype(np.float64)
    w["iota128"] = np.broadcast_to(np.arange(128, dtype=np.float32), (128, 128)).copy()
    w["iota32"] = np.broadcast_to(np.arange(32, dtype=np.float32), (128, 32)).copy()
    w["ident"] = np.eye(128, dtype=np.float32).astype(bfloat16)
    Wg1 = inp["Wg1"].astype(np.float32)
    w["wg1_h1"] = np.concatenate([Wg1[:HID], inp["bg1"].astype(np.float32)[None, :]], 0).astype(bfloat16)   # [161,160]
    w["wg1_h2"] = np.concatenate([Wg1[HID:], np.zeros((1, HID), np.float32)], 0).astype(bfloat16)           # [161,160]
    w["wg2rep"] = np.broadcast_to(inp["Wg2"].astype(np.float32)[:, 0], (128, HID)).astype(bfloat16)
    w["wh1"] = np.concatenate([inp["Wh1"].astype(np.float32), inp["bh1"].astype(np.float32)[None, :]], 0).astype(bfloat16)  # [321,320]
    w["wh2"] = np.concatenate([inp["Wh2"].astype(np.float32), inp["bh2"].astype(np.float32)[None, :]], 0).astype(bfloat16)  # [321,6]
    w["win"] = inp["W_in"].astype(np.float32).astype(bfloat16)  # [5,160]
    w["bg2rep"] = np.full((128, 1), float(np.asarray(inp["bg2"]).reshape(-1)[0]), np.float32)
    for layer in range(2):
        Wq, Wk, Wv = (inp[k][layer].astype(np.float64) for k in ("Wq", "Wk", "Wv"))
        bq, bk, bv = (inp[k][layer].astype(np.float64) for k in ("bq", "bk", "bv"))
        Wskip, bskip = inp["Wskip"][layer].astype(np.float64), inp["bskip"][layer].astype(np.float64)
        Wbeta = inp["Wbeta"][layer].astype(np.float64)
        We, be = inp["We"][layer].astype(np.float64), inp["be"][layer].astype(np.float64)
        if layer == 0:
            bq, bk, bv, bskip = bq + b_in @ Wq, bk + b_in @ Wk, bv + b_in @ Wv, bskip + b_in @ Wskip
        P = (Wbeta[:HID, 0] + Wbeta[2 * HID :, 0])
        Q = (Wbeta[HID : 2 * HID, 0] - Wbeta[2 * HID :, 0])
        We5 = np.concatenate([We, be[None, :]], 0)                     # [5,160]
        # kv table row: [k(160) | v(160)] (+64 cols dram pad to 384)
        Wkf = np.concatenate([Wk, bk[None, :]], 0)                     # [161,160]
        Wvf = np.concatenate([Wv, bv[None, :]], 0)
        w[f"wkv{layer}"] = np.concatenate([Wkf, Wvf], 1).astype(np.float32).astype(bfloat16)  # [161,320]
        # k-side e-term weight (alpha): e_k = [ea,1] @ We5
        w[f"wek{layer}"] = We5.astype(np.float32).astype(bfloat16)     # [5,160]
        # merged q | r | -rbQ weight: [161, 321]. col 320 holds MINUS the
        # (skip @ Q) fold = bias of exp(-z) in the exp-form sigmoid.
        Wqf = np.concatenate([Wq, bq[None, :]], 0)                     # [161,160]
        qrb = np.zeros((161, 321))
        qrb[:, 0:160] = Wqf
        qrb[:160, 160:320] = Wskip
        qrb[160, 160:320] = bskip
        qrb[:160, 320] = -(Wskip @ Q)
        qrb[160, 320] = -(bskip @ Q)
        w[f"wqrb{layer}"] = qrb.astype(np.float32).astype(bfloat16)    # [161,321]
        # scatter e-fold: acc_ea [n,(h,j)] @ We5BD -> per-head v e-term [n,160]
        we5bd = np.zeros((20, HID))
        for h in range(H):
            we5bd[5 * h : 5 * h + 5, 40 * h : 40 * h + 40] = We5[:, 40 * h : 40 * h + 40]
        w[f"we5bd{layer}"] = we5bd.astype(np.float32).astype(bfloat16)  # [20,160]
        w[f"prep{layer}"] = np.broadcast_to(P.astype(np.float32), (128, HID)).copy()
    return w


def _build(C_L, C_H, phases="full", winlim=None, skips=()):
    skips = set(skips)
    NCH = C_L + C_H
    NGL = (WIN * C_L * 128 + GS - 1) // GS
    NGH = (WIN * C_H * 128 + GS - 1) // GS

    nc = bacc.Bacc(get_trn_type() or "TRN2", target_bir_lowering=False, num_swdge_queues=2)

    # ---- dram I/O ----
    d = {}
    d["xT"] = nc.dram_tensor("xT", [5, NSHARD], bf16, kind="ExternalInput")
    d["idxL"] = nc.dram_tensor("idxL", [128, NGL * GS // 16], i16, kind="ExternalInput")
    d["idxH"] = nc.dram_tensor("idxH", [128, NGH * GS // 16], i16, kind="ExternalInput")
    d["sse"] = nc.dram_tensor("sse", [WIN, 128, NCH * 276], bf16, kind="ExternalInput")
    d["eaT"] = nc.dram_tensor("eaT", [WIN, 5, NCH * 128], bf16, kind="ExternalInput")
    d["batchc"] = nc.dram_tensor("batchc", [WIN, 128, 1], f32, kind="ExternalInput")
    wshapes = dict(
        iota128=([128, 128], f32), iota32=([128, 32], f32), ident=([128, 128], bf16),
        wg1_h1=([161, HID], bf16), wg1_h2=([161, HID], bf16), wg2rep=([128, HID], bf16),
        wh1=([321, JK], bf16), wh2=([321, 6], bf16), win=([5, HID], bf16),
        bg2rep=([128, 1], f32),
    )
    for layer in range(2):
        wshapes[f"wkv{layer}"] = ([161, JK], bf16)
        wshapes[f"wek{layer}"] = ([5, HID], bf16)
        wshapes[f"wqrb{layer}"] = ([161, 321], bf16)
        wshapes[f"prep{layer}"] = ([128, HID], f32)
        wshapes[f"we5bd{layer}"] = ([20, HID], bf16)
    for k, (shp, dt) in wshapes.items():
        d[k] = nc.dram_tensor(k, shp, dt, kind="ExternalInput")
    out_d = nc.dram_tensor("out", [32, 6], f32, kind="ExternalOutput")
    dbg_d = nc.dram_tensor("dbg", [128, JK], bf16, kind="ExternalOutput")

    hT = [nc.dram_tensor(f"hT{i}", [HID, NSHARD], bf16) for i in range(3)]
    h_nm = [None, nc.dram_tensor("h_nm1", [NSHARD, HID], bf16),
            nc.dram_tensor("h_nm2", [NSHARD, HID], bf16)]
    kv_own = [nc.dram_tensor(f"kv_own{l}", [NSHARD, KVP], bf16) for l in range(2)]
    kv_full = [nc.dram_tensor(f"kv_full{l}", [NPAD, KVP], bf16, addr_space="Shared")
               for l in range(2)]
    pool_in = nc.dram_tensor("pool_in", [32, JK + 1], f32)
    pool_out = nc.dram_tensor("pool_out", [32, JK + 1], f32, addr_space="Shared")
    rg = [list(range(NCORES))]

    with tile.TileContext(nc) as tc:
        with (
            tc.tile_pool(name="const", bufs=1) as cst,
            tc.tile_pool(name="sb", bufs=2) as sb,
            tc.tile_pool(name="gath", bufs=3) as gath,
            tc.tile_pool(name="ps", bufs=2, space="PSUM") as ps,
        ):
            nc.gpsimd.load_library(mlp)
            KW = 1
            WTAIL = WIN % KW if WIN % KW else KW
            regs = {("L", KW): nc.gpsimd.to_reg(KW * C_L * 128),
                    ("H", KW): nc.gpsimd.to_reg(KW * C_H * 128)}
            if WTAIL != KW:
                regs[("L", WTAIL)] = nc.gpsimd.to_reg(WTAIL * C_L * 128)
                regs[("H", WTAIL)] = nc.gpsimd.to_reg(WTAIL * C_H * 128)

            # ---- persistent constants ----
            C = {}
            def _load_const(key, part, cols, row0=0):
                t = cst.tile([part, cols], wshapes[key][1], name=f"c_{key}_{row0}")
                nc.sync.dma_start(out=t[:], in_=d[key][row0 : row0 + part, :])
                return t
            for layer in range(2):
                C[f"wkv{layer}a"] = _load_const(f"wkv{layer}", 128, JK)
                C[f"wkv{layer}b"] = _load_const(f"wkv{layer}", 32, JK, 128)
                C[f"wkv{layer}c"] = _load_const(f"wkv{layer}", 1, JK, 160)
                C[f"wek{layer}"] = _load_const(f"wek{layer}", 5, HID)
                C[f"wqrb{layer}a"] = _load_const(f"wqrb{layer}", 128, 321)
                C[f"wqrb{layer}b"] = _load_const(f"wqrb{layer}", 32, 321, 128)
                C[f"wqrb{layer}c"] = _load_const(f"wqrb{layer}", 1, 321, 160)
                C[f"prep{layer}"] = _load_const(f"prep{layer}", 128, HID)
                C[f"we5bd{layer}"] = _load_const(f"we5bd{layer}", 20, HID)
            C["iota128"] = _load_const("iota128", 128, 128)
            C["iota32"] = _load_const("iota32", 128, 32)
            C["ident"] = _load_const("ident", 128, 128)
            C["wg2rep"] = _load_const("wg2rep", 128, HID)
            for key in ("wg1_h1", "wg1_h2"):
                C[key + "a"] = _load_const(key, 128, HID)
                C[key + "b"] = _load_const(key, 32, HID, 128)
                C[key + "c"] = _load_const(key, 1, HID, 160)
            C["bg2rep"] = _load_const("bg2rep", 128, 1)
            C["wh1a"] = _load_const("wh1", 128, JK)
            C["wh1b"] = _load_const("wh1", 128, JK, 128)
            C["wh1c"] = _load_const("wh1", 64, JK, 256)
            C["wh1d"] = _load_const("wh1", 1, JK, 320)
            C["wh2a"] = _load_const("wh2", 128, 6)
            C["wh2b"] = _load_const("wh2", 128, 6, 128)
            C["wh2c"] = _load_const("wh2", 64, 6, 256)
            C["wh2d"] = _load_const("wh2", 1, 6, 320)
            C["win"] = _load_const("win", 5, HID)

            idxLt = cst.tile([128, NGL * GS // 16], i16, name="idxLt")
            nc.sync.dma_start(out=idxLt[:], in_=d["idxL"][:])
            idxHt = cst.tile([128, NGH * GS // 16], i16, name="idxHt")
            nc.sync.dma_start(out=idxHt[:], in_=d["idxH"][:])

            # ---- phase 0: h0T = (x @ W_in)^T, own nodes ----
            with nc.named_scope("p0"):
                NT0 = (NSHARD + 511) // 512
                for t in range(NT0):
                    c0, cn = t * 512, min(512, NSHARD - t * 512)
                    xts = sb.tile([5, cn], bf16, tag="xts")
                    nc.sync.dma_start(out=xts[:], in_=d["xT"][:, c0 : c0 + cn])
                    for (r0, m) in ((0, 128), (128, 32)):
                        ph = ps.tile([m, cn], f32, tag="kve", bufs=2)
                        nc.tensor.matmul(ph[:], C["win"][:, r0 : r0 + m], xts[:],
                                         start=True, stop=True)
                        hsb = sb.tile([m, cn], bf16, tag="hsb")
                        nc.vector.tensor_copy(out=hsb[:], in_=ph[:])
                        nc.sync.dma_start(out=hT[0][r0 : r0 + m, c0 : c0 + cn], in_=hsb[:])

            ones1 = cst.tile([1, 128], bf16, name="ones1")
            nc.gpsimd.memset(ones1[:], 1.0)

            # ==== two layers ====
            nlayers = 0 if phases == "p0" else (1 if phases in ("kv", "edge0") else 2)
            for layer in range(nlayers):
                hsrc = hT[layer]
                # ---- kv GEMM own nodes -> kv_own ----
                with nc.named_scope(f"kv{layer}"):
                    for t in range(WIN):
                        csl = slice(t * 128, (t + 1) * 128)
                        hta = sb.tile([128, 128], bf16, tag="hta", bufs=3)
                        nc.sync.dma_start(out=hta[:], in_=hsrc[0:128, csl])
                        htb = sb.tile([32, 128], bf16, tag="htb", bufs=3)
                        nc.sync.dma_start(out=htb[:], in_=hsrc[128:160, csl])
                        pkv = ps.tile([128, JK], f32, tag="kve", bufs=2)
                        nc.tensor.matmul(pkv[:], hta[:], C[f"wkv{layer}a"][:], start=True, stop=False)
                        nc.tensor.matmul(pkv[:], htb[:], C[f"wkv{layer}b"][:], start=False, stop=False)
                        nc.tensor.matmul(pkv[:], ones1[:, :128], C[f"wkv{layer}c"][:], start=False, stop=True)
                        kvsb = sb.tile([128, JK], bf16, tag="kvsb")
                        nc.vector.tensor_copy(out=kvsb[:], in_=pkv[:])
                        nc.sync.dma_start(out=kv_own[layer][csl, 0:JK], in_=kvsb[:])
                with nc.named_scope(f"ag{layer}"):
                    nc.gpsimd.collective_compute(
                        "AllGather", mybir.AluOpType.bypass, replica_groups=rg,
                        ins=[kv_own[layer][:]], outs=[kv_full[layer][:]])

                # ---- edge phase ----
                if phases == "kv":
                    break
                with nc.named_scope(f"edge{layer}"):
                    def _gather(region, w0):
                        idxt, base, CC = (
                            (idxLt, 0, C_L) if region == "L" else (idxHt, SPLIT, C_H)
                        )
                        nw = min(KW, WIN - w0)
                        gtile = gath.tile([128, KW * CC, KVP], bf16, tag="g" + region)
                        nc.gpsimd.dma_gather(
                            gtile[:, 0 : nw * CC, :],
                            kv_full[layer][base : base + SPLIT, :],
                            idxt[:, w0 * (CC * 8) : (w0 + nw) * (CC * 8)],
                            num_idxs=nw * CC * 128,
                            num_idxs_reg=regs[(region, nw)],
                            elem_size=KVP,
                            queue_num=(0 if region == "L" else 1))
                        return gtile

                    gtL = gtH = None
                    for w in range(WIN if winlim is None else winlim):
                        wsl = slice(w * 128, (w + 1) * 128)
                        if w % KW == 0:
                            gtL = _gather("L", w)
                            gtH = _gather("H", w)
                        wo = w % KW
                        ssew = sb.tile([128, NCH * 276], bf16, tag="ssew", bufs=2)
                        nc.scalar.dma_start(out=ssew[:], in_=d["sse"][w])
                        stw = ssew[:, 0 : NCH * 128]
                        ssw = ssew[:, NCH * 128 : NCH * 256]
                        earw = ssew[:, NCH * 256 : NCH * 276]
                        hta = sb.tile([128, 128], bf16, tag="hta", bufs=3)
                        nc.sync.dma_start(out=hta[:], in_=hsrc[0:128, wsl])
                        htb = sb.tile([32, 128], bf16, tag="htb", bufs=3)
                        nc.sync.dma_start(out=htb[:], in_=hsrc[128:160, wsl])
                        eaw = sb.tile([5, NCH * 128], bf16, tag="eaw", bufs=3)
                        nc.scalar.dma_start(out=eaw[:], in_=d["eaT"][w])
                        # q | r | -rbQ for this window (one merged GEMM)
                        pq = ps.tile([128, 321], f32, tag="qrb", bufs=1)
                        nc.tensor.matmul(pq[:], hta[:], C[f"wqrb{layer}a"][:], start=True, stop=False)
                        nc.tensor.matmul(pq[:], htb[:], C[f"wqrb{layer}b"][:], start=False, stop=False)
                        nc.tensor.matmul(pq[:], ones1[:, :128], C[f"wqrb{layer}c"][:], start=False, stop=True)
                        qw = sb.tile([128, HID], bf16, tag="qw", bufs=3)
                        nc.scalar.activation(out=qw[:], in_=pq[:, 0:HID],
                                             func=mybir.ActivationFunctionType.Copy)
                        rsb = sb.tile([128, 161], f32, tag="rsb", bufs=3)
                        nc.scalar.activation(out=rsb[:], in_=pq[:, HID:321],
                                             func=mybir.ActivationFunctionType.Copy)

                        pacc = ps.tile([128, 184], f32, tag="acc")
                        for j in range(NCH):
                            if j < C_L:
                                gt, cc = gtL, wo * C_L + j
                            else:
                                gt, cc = gtH, wo * C_H + (j - C_L)
                            ear = earw[:, j * 20 : (j + 1) * 20].rearrange(
                                "p (h f) -> p h f", h=H, f=5)
                            # k-side e-term + k_e = k_g + e_k (bf16 out)
                            pek = ps.tile([128, HID], f32, tag="kve", bufs=2)
                            nc.tensor.matmul(pek[:], eaw[:, j * 128 : (j + 1) * 128],
                                             C[f"wek{layer}"][:], start=True, stop=True)
                            ek16 = sb.tile([128, HID], bf16, tag="ek16", bufs=6)
                            nc.scalar.activation(out=ek16[:], in_=pek[:],
                                                 func=mybir.ActivationFunctionType.Copy)
                            ke16 = sb.tile([128, HID], bf16, tag="ke16", bufs=6)
                            nc.vector.tensor_tensor(out=ke16[:], in0=ek16[:],
                                                    in1=gt[:, cc, 0:HID],
                                                    op=mybir.AluOpType.add)
                            # q gathered to edges; PSUM->SBUF bf16 copy on ACT
                            pqg = ps.tile([128, HID], f32, tag="tp", bufs=3)
                            nc.tensor.matmul(pqg[:], ssw[:, j * 128 : (j + 1) * 128],
                                             qw[:], start=True, stop=True,
                                             skip_group_check=True)
                            qg = sb.tile([128, HID], bf16, tag="qg", bufs=6)
                            nc.scalar.activation(out=qg[:], in_=pqg[:],
                                                 func=mybir.ActivationFunctionType.Copy)
                            # alpha partial products (bf16 2x) + per-head reduce
                            tq = sb.tile([128, HID], bf16, tag="tq", bufs=6)
                            nc.vector.tensor_tensor(out=tq[:], in0=qg[:],
                                                    in1=ke16[:],
                                                    op=mybir.AluOpType.mult)
                            al = sb.tile([128, H], f32, tag="al", bufs=6)
                            nc.vector.tensor_reduce(
                                out=al[:],
                                in_=tq[:].rearrange("p (h f) -> p h f", h=H),
                                axis=mybir.AxisListType.X, op=mybir.AluOpType.add)
                            # wt payload: [v*ex (160) | ea5*ex (20) | ex (4)]
                            wt = sb.tile([128, 184], bf16, tag="wt", bufs=6)
                            nc.scalar.activation(
                                out=wt[:, 180:184], in_=al[:],
                                func=mybir.ActivationFunctionType.Exp, scale=INVSQD)
                            nc.vector.tensor_tensor(
                                out=wt[:, 0:160].rearrange("p (h dd) -> p h dd", h=H),
                                in0=gt[:, cc, HID:JK].rearrange("p (h dd) -> p h dd", h=H),
                                in1=wt[:, 180:184].rearrange("p (h o) -> p h o", h=H).to_broadcast([128, H, D]),
                                op=mybir.AluOpType.mult)
                            nc.vector.tensor_tensor(
                                out=wt[:, 160:180].rearrange("p (h f) -> p h f", h=H),
                                in0=ear,
                                in1=wt[:, 180:184].rearrange("p (h o) -> p h o", h=H).to_broadcast([128, H, 5]),
                                op=mybir.AluOpType.mult)
                            # scatter: acc[nodes] += S^T.T @ wt
                            nc.tensor.matmul(pacc[:], stw[:, j * 128 : (j + 1) * 128],
                                             wt[:],
                                             start=(j == 0), stop=(j == NCH - 1),
                                             skip_group_check=True)

                        # ---- window post: out = num/den, beta gate, h' ----
                        accsb = sb.tile([128, 184], f32, tag="accsb")
                        nc.vector.tensor_copy(out=accsb[:], in_=pacc[:])
                        dmax = sb.tile([128, H], f32, tag="dmax")
                        nc.vector.tensor_scalar_max(dmax[:], accsb[:, 180:184], 1e-30)
                        denr = sb.tile([128, H], f32, tag="denr")
                        nc.vector.reciprocal(out=denr[:], in_=dmax[:])
                        # v e-term: (acc_ea [n,(h,j)] @ We5BD) added to acc_v
                        ab16 = sb.tile([128, 20], bf16, tag="ab16")
                        nc.vector.tensor_copy(out=ab16[:], in_=accsb[:, 160:180])
                        ptre = ps.tile([20, 128], bf16, tag="tp", bufs=3)
                        nc.tensor.transpose(ptre[:], ab16[:], C["ident"][:])
                        aet = sb.tile([20, 128], bf16, tag="aet")
                        nc.vector.tensor_copy(out=aet[:], in_=ptre[:])
                        pve = ps.tile([128, HID], f32, tag="tp", bufs=3)
                        nc.tensor.matmul(pve[:], aet[:], C[f"we5bd{layer}"][:],
                                         start=True, stop=True, skip_group_check=True)
                        pv16 = sb.tile([128, HID], bf16, tag="pv16")
                        nc.scalar.activation(out=pv16[:], in_=pve[:],
                                             func=mybir.ActivationFunctionType.Copy)
                        vsum = sb.tile([128, HID], f32, tag="vsum")
                        nc.vector.tensor_tensor(out=vsum[:], in0=pv16[:],
                                                in1=accsb[:, 0:160],
                                                op=mybir.AluOpType.add)
                        outn = sb.tile([128, HID], f32, tag="outn")
                        nc.vector.tensor_tensor(
                            out=outn[:].rearrange("p (h dd) -> p h dd", h=H),
                            in0=vsum[:].rearrange("p (h dd) -> p h dd", h=H),
                            in1=denr[:].rearrange("p (h o) -> p h o", h=H).to_broadcast([128, H, D]),
                            op=mybir.AluOpType.mult)
                        scr = sb.tile([128, HID], f32, tag="scr")
                        outP = sb.tile([128, 1], f32, tag="outP")
                        nc.vector.tensor_tensor(out=scr[:], in0=outn[:],
                            in1=C[f"prep{layer}"][:], op=mybir.AluOpType.mult)
                        nc.vector.tensor_reduce(out=outP[:],
                            in_=scr[:].rearrange("p (a b) -> p a b", a=1),
                            axis=mybir.AxisListType.XY, op=mybir.AluOpType.add)
                        # beta = sigmoid(outP + rb) computed via the Exp table:
                        # texp = exp(-outP - rb)  (rsb col 160 already holds -rb)
                        # beta = 1 / (1 + texp)
                        beta = sb.tile([128, 1], f32, tag="beta")
                        if "sig" not in skips:
                            texp = sb.tile([128, 1], f32, tag="texp")
                            nc.scalar.activation(out=texp[:], in_=outP[:],
                                                 func=mybir.ActivationFunctionType.Exp,
                                                 bias=rsb[:, 160:161], scale=-1.0)
                            u1 = sb.tile([128, 1], f32, tag="u1")
                            nc.vector.tensor_scalar_add(u1[:], texp[:], 1.0)
                            nc.vector.reciprocal(out=beta[:], in_=u1[:])
                        else:
                            nc.vector.tensor_copy(out=beta[:], in_=outP[:])
                        dvec = sb.tile([128, HID], f32, tag="dvec")
                        nc.vector.tensor_sub(dvec[:], rsb[:, :HID], outn[:])
                        hp = sb.tile([128, HID], bf16, tag="hp")
                        if "stt" not in skips:
                            nc.vector.scalar_tensor_tensor(
                                out=hp[:], in0=dvec[:], scalar=beta[:, 0:1], in1=outn[:],
                                op0=mybir.AluOpType.mult, op1=mybir.AluOpType.add)
                        else:
                            nc.vector.tensor_scalar_mul(hp[:], dvec[:], beta[:, 0:1])
                            nc.vector.tensor_add(hp[:], hp[:], outn[:])
                        nc.sync.dma_start(out=h_nm[layer + 1][wsl, :], in_=hp[:])
                        # transpose h' into hT[layer+1]
                        if "trans" in skips:
                            continue
                        ptr1 = ps.tile([128, 128], bf16, tag="tp", bufs=3)
                        nc.tensor.transpose(ptr1[:], hp[:, 0:128], C["ident"][:])
                        t1 = sb.tile([128, 128], bf16, tag="t1")
                        nc.vector.tensor_copy(out=t1[:], in_=ptr1[:])
                        nc.sync.dma_start(out=hT[layer + 1][0:128, wsl], in_=t1[:])
                        ptr2 = ps.tile([32, 128], bf16, tag="tp", bufs=3)
                        nc.tensor.transpose(ptr2[:], hp[:, 128:160], C["ident"][:])
                        t2 = sb.tile([32, 128], bf16, tag="t2")
                        nc.vector.tensor_copy(out=t2[:], in_=ptr2[:])
                        nc.sync.dma_start(out=hT[layer + 1][128:160, wsl], in_=t2[:])

            if phases == "p0":
                dbgt = sb.tile([128, JK], bf16, tag="dbgt")
                nc.gpsimd.memset(dbgt[:], 0.0)
                nc.sync.dma_start(out=dbgt[:, :160], in_=hT[0][0:128, 999:1159])
                nc.sync.dma_start(out=dbg_d[:], in_=dbgt[:])
            # ==== final phase: gate + graph pooling + head MLP ====
            if phases != "full":
                dummy = sb.tile([32, 6], f32, tag="osb")
                nc.gpsimd.memset(dummy[:], 0.0)
                nc.sync.dma_start(out=out_d[:], in_=dummy[:])
            if phases == "full":
              with nc.named_scope("final"):
                pgr = ps.tile([32, JK + 1], f32, tag="acc")
                for w in range(WIN):
                    wsl = slice(w * 128, (w + 1) * 128)
                    h1w = sb.tile([128, HID], bf16, tag="h1w")
                    nc.sync.dma_start(out=h1w[:], in_=h_nm[1][wsl, :])
                    h2w = sb.tile([128, HID], bf16, tag="h2w")
                    nc.sync.dma_start(out=h2w[:], in_=h_nm[2][wsl, :])
                    bcw = sb.tile([128, 1], f32, tag="bcw")
                    nc.sync.dma_start(out=bcw[:], in_=d["batchc"][w])
                    pg = ps.tile([128, HID], f32, tag="kve", bufs=2)
                    first = True
                    for (src_hT, wkey) in ((hT[1], "wg1_h1"), (hT[2], "wg1_h2")):
                        g_a = sb.tile([128, 128], bf16, tag="hta", bufs=3)
                        nc.sync.dma_start(out=g_a[:], in_=src_hT[0:128, wsl])
                        g_b = sb.tile([32, 128], bf16, tag="htb", bufs=3)
                        nc.sync.dma_start(out=g_b[:], in_=src_hT[128:160, wsl])
                        nc.tensor.matmul(pg[:], g_a[:], C[wkey + "a"][:], start=first, stop=False)
                        first = False
                        nc.tensor.matmul(pg[:], g_b[:], C[wkey + "b"][:], start=False, stop=False)
                    nc.tensor.matmul(pg[:], ones1[:, :128], C["wg1_h1c"][:], start=False, stop=True)
                    grelu = sb.tile([128, HID], bf16, tag="grelu")
                    nc.scalar.activation(out=grelu[:], in_=pg[:],
                                         func=mybir.ActivationFunctionType.Relu)
                    scr2 = sb.tile([128, HID], bf16, tag="scr")
                    gatec = sb.tile([128, 1], f32, tag="gatec")
                    nc.vector.tensor_tensor(out=scr2[:], in0=grelu[:],
                        in1=C["wg2rep"][:], op=mybir.AluOpType.mult)
                    nc.vector.tensor_reduce(out=gatec[:],
                        in_=scr2[:].rearrange("p (a b) -> p a b", a=1),
                        axis=mybir.AxisListType.XY, op=mybir.AluOpType.add)
                    ge = sb.tile([128, 1], f32, tag="ge")
                    nc.scalar.activation(out=ge[:], in_=gatec[:],
                                         func=mybir.ActivationFunctionType.Exp,
                                         bias=C["bg2rep"][:, 0:1])
                    sg = sb.tile([128, 32], bf16, tag="sg")
                    nc.vector.tensor_tensor(out=sg[:], in0=bcw[:].to_broadcast([128, 32]),
                                            in1=C["iota32"][:], op=mybir.AluOpType.is_equal)
                    wg = sb.tile([128, JK + 1], bf16, tag="wg")
                    nc.vector.tensor_scalar_mul(wg[:, 0:HID], h1w[:], ge[:, 0:1])
                    nc.vector.tensor_scalar_mul(wg[:, HID:JK], h2w[:], ge[:, 0:1])
                    nc.vector.tensor_copy(out=wg[:, JK : JK + 1], in_=ge[:])
                    nc.tensor.matmul(pgr[:], sg[:], wg[:], start=(w == 0),
                                     stop=(w == WIN - 1), skip_group_check=True)
                pg_sb = sb.tile([32, JK + 1], f32, tag="pg_sb")
                nc.vector.tensor_copy(out=pg_sb[:], in_=pgr[:])
                nc.sync.dma_start(out=pool_in[:], in_=pg_sb[:])
                nc.gpsimd.collective_compute(
                    "AllReduce", mybir.AluOpType.add, replica_groups=rg,
                    ins=[pool_in[:]], outs=[pool_out[:]])
                psb = sb.tile([32, JK + 1], f32, tag="psb")
                nc.sync.dma_start(out=psb[:], in_=pool_out[:])
                gden = sb.tile([32, 1], f32, tag="gden")
                nc.vector.tensor_scalar_max(gden[:], psb[:, JK : JK + 1], 1e-30)
                gdr = sb.tile([32, 1], f32, tag="gdr")
                nc.vector.reciprocal(out=gdr[:], in_=gden[:])
                pl = sb.tile([32, JK], bf16, tag="pl")
                nc.vector.tensor_scalar_mul(pl[:], psb[:, 0:JK], gdr[:, 0:1])

                def _headmm(vin, wa, wb, wc, wd, nout, tagp):
                    """vin [32, 320] @ W[320, nout] + bias via PE transposes."""
                    pouts = ps.tile([32, nout], f32, tag=tagp, bufs=(2 if tagp == "kve" else 1))
                    for si, (c0, m) in enumerate(((0, 128), (128, 128), (256, 64))):
                        ptt = ps.tile([m, 32], bf16, tag="tp", bufs=3)
                        nc.tensor.transpose(ptt[:], vin[:, c0 : c0 + m], C["ident"][0:32, 0:32])
                        tsb = sb.tile([m, 32], bf16, tag="tsb")
                        nc.vector.tensor_copy(out=tsb[:], in_=ptt[:])
                        nc.tensor.matmul(pouts[:], tsb[:], (wa, wb, wc)[si][:m, :],
                                         start=(si == 0), stop=False, skip_group_check=True)
                    nc.tensor.matmul(pouts[:], ones1[:, :32], wd[:],
                                     start=False, stop=True, skip_group_check=True)
                    return pouts

                ph1 = _headmm(pl, C["wh1a"], C["wh1b"], C["wh1c"], C["wh1d"], JK, "qrb")
                vrel = sb.tile([32, JK], bf16, tag="vrel")
                nc.scalar.activation(out=vrel[:], in_=ph1[:],
                                     func=mybir.ActivationFunctionType.Relu)
                ph2 = _headmm(vrel, C["wh2a"], C["wh2b"], C["wh2c"], C["wh2d"], 6, "kve")
                osb = sb.tile([32, 6], f32, tag="osb")
                nc.vector.tensor_copy(out=osb[:], in_=ph2[:])
                nc.sync.dma_start(out=out_d[:], in_=osb[:])

    nc.compile()
    return nc


_CACHE = {}
_LAST_RES = None


def kernel(**inputs):
    inputs = {k: np.asarray(v) for k, v in inputs.items()}
    per_core, C_L, C_H = _preprocess(
        inputs["x"], inputs["edge_index"], inputs["edge_attr"], inputs["batch"])
    w = _weights(inputs)
    import os as _os
    phases = _os.environ.get("KERNEL_PHASES", "full")
    winlim = _os.environ.get("KERNEL_WINLIM")
    winlim = int(winlim) if winlim else None
    skips = tuple(s for s in _os.environ.get("KERNEL_SKIP", "").split(",") if s)
    key = (C_L, C_H, phases, winlim, skips)
    if key not in _CACHE:
        _CACHE[key] = _build(C_L, C_H, phases, winlim, skips)
    nc = _CACHE[key]
    in_maps = []
    for r in range(NCORES):
        m = dict(w)
        m.update(per_core[r])
        in_maps.append(m)
    import os
    trace = bool(os.environ.get("KERNEL_TRACE"))
    if trace:
        try:
            import antenv.axon_hooks  # noqa: F401
        except ImportError:
            try:
                import sys as _sys
                import types as _types
                import antenv as _antenv
                from trn_agent_boot.trn_boot import _ntff_profile_via_ctypes
                _m = _types.ModuleType("antenv.axon_hooks")
                _m._hook = _ntff_profile_via_ctypes("/opt/axon/libaxon_pjrt.so")
                _m.set_axon_ntff_profile_hook = lambda h: setattr(_m, "_hook", h)
                _m.get_axon_ntff_profile_hook = lambda: _m._hook
                _sys.modules["antenv.axon_hooks"] = _m
                _antenv.axon_hooks = _m
            except Exception:
                trace = False
    res = run_bass_kernel_spmd(nc, in_maps, core_ids=list(range(NCORES)), trace=trace)
    if trace and res.exec_time_ns is not None:
        print(f"HW exec time: {res.exec_time_ns} ns")
        if res.per_core_scope_times:
            for scope, cores in sorted(res.per_core_scope_times.items()):
                print(f"  scope {scope}: {cores}")
    global _LAST_RES
    _LAST_RES = res
    out = res.results[0]["out"]
    return out.reshape(G, 2, 3).astype(np.float32)
